# revision 34
# baseline (speedup 1.0000x reference)
"""Gemma2 attention (B=2, S=2048, HID=2304, H=8, KVH=4, D=256, window=1024,
softcap=50) on 8 TRN2 NeuronCores.

Sharding: DP2 (batch) x TP4 (heads). Core c -> batch c//4, TP rank r=c%4 with
Q heads {2r, 2r+1} and KV head r (GQA-aligned). Wo is row-split over the head
dim; the 4 partial outputs per batch are summed on the host.

Device kernel v2 (identical program on all cores):
  - Scores are computed TRANSPOSED ([keys, queries]) for query-block PAIRS
    (256 queries wide), so the softmax weights come out of the exp already in
    the layout AV needs as the stationary operand -- no PE transposes of P.
  - No rowmax: weights = exp(50*tanh(s/50) - 50) <= 1 stored in bf16, whose
    dynamic range (down to ~1e-38) covers any realizable row maximum.
  - Row sums come from a ones-column appended to V (AV matmuls are N=257);
    1/rowsum is folded into the AV eviction (per-partition scalar).
  - Masks are additive -3 tiles applied pre-exp (exp then underflows to 0).
  - Host-side input relayout gives every DMA 128 large contiguous
    per-partition descriptors; output partials are fp16.
  - Dummy matmuls warm the PE (HAM un-throttle) during the initial DMA wait.
"""
import sys

import numpy as np

try:
    import concourse.bass  # noqa: F401
except ImportError:
    sys.path.insert(0, "/opt/trn_rl_repo")

H, KVH, D = 8, 4, 256
S, HID = 2048, 2304
B = 2
SCALING = 256.0 ** -0.5
SOFTCAP = 50.0
THETA = 10000.0
WINDOW = 1024

P = 128
KC = HID // P            # 18 contraction chunks for projections
NTC = 4                  # token chunks for projections
TCW = S // NTC           # 512
NPAIR = 8                # query-block pairs (256 queries each)
HG_WIDTHS = [512, 512, 512, 512, 256]   # 2304 split for Wo output groups

_CACHED = {}


def _build_nc():
    import concourse.bass as bass
    import concourse.mybir as mybir
    import concourse.tile as tile
    from concourse import bacc
    from concourse.masks import make_identity

    f32 = mybir.dt.float32
    f16 = mybir.dt.float16
    bf16 = mybir.dt.bfloat16
    AF = mybir.ActivationFunctionType

    nc = bacc.Bacc(None, target_bir_lowering=False)

    h4 = nc.dram_tensor("h4", [P, NTC, KC, TCW], f16, kind="ExternalInput")
    wq4 = nc.dram_tensor("wq4", [P, KC, 2 * D], f16, kind="ExternalInput")
    wk4 = nc.dram_tensor("wk4", [P, KC, D], f16, kind="ExternalInput")
    wv4 = nc.dram_tensor("wv4", [P, KC, D], f16, kind="ExternalInput")
    wo4 = nc.dram_tensor("wo4", [P, 4, HID], bf16, kind="ExternalInput")
    cosT = nc.dram_tensor("cosT", [P, S], f16, kind="ExternalInput")
    sinT = nc.dram_tensor("sinT", [P, S], f16, kind="ExternalInput")
    out = nc.dram_tensor("out", [S, HID], f16, kind="ExternalOutput")

    with tile.TileContext(nc) as tc:
        with (
            tc.tile_pool(name="wpool", bufs=1) as wpool,
            tc.tile_pool(name="hpool", bufs=2) as hpool,
            tc.tile_pool(name="qkv", bufs=1) as qkv,
            tc.tile_pool(name="work", bufs=2) as work,
            tc.tile_pool(name="att3", bufs=4) as att3,
            tc.tile_pool(name="sc", bufs=1) as scpool,
            tc.tile_pool(name="ptp", bufs=2) as ptpool,
            tc.tile_pool(name="sm", bufs=2) as smpool,
            tc.tile_pool(name="psA", bufs=8, space="PSUM") as psA,
        ):
            # ---------------- persistent SBUF ----------------
            wq_sb = wpool.tile([P, KC, 2 * D], f16)
            wk_sb = wpool.tile([P, KC, D], f16)
            wv_sb = wpool.tile([P, KC, D], f16)
            wo_sb = wpool.tile([P, 4, HID], bf16)
            cos_sb = wpool.tile([P, S], f16)
            sin_sb = wpool.tile([P, S], f16)
            ident_bf = wpool.tile([P, P], bf16)
            wtile = wpool.tile([P, P], f16)
            negb = wpool.tile([P, 1], f32)
            mA = wpool.tile([P, 2, 256], f32)
            mB = wpool.tile([P, 2, 256], f32)
            mC = wpool.tile([P, 2, 256], f32)
            mD = wpool.tile([P, 2, 256], f32)

            qt_sb = qkv.tile([P, 4, S], f16)      # QT feature-major
            kt_sb = qkv.tile([P, 2, S], f16)      # KT feature-major
            vE_sb = qkv.tile([P, 16, D + 1], bf16)  # V token-major + ones col

            # DMA: large per-partition-contiguous descriptors, issued in
            # rounds of k-chunks so the ts=0 k-outer projection pass can
            # consume chunk k as soon as its round lands.
            ht0 = hpool.tile([P, KC, TCW], f16, tag="ht", name="ht0")
            # cos/sin go after the projection operands: rope first needs them
            # only at the end of the ts=0 k-outer pass (~20us later), and
            # issuing them mid-stream delays the later ht/w rounds.
            for a, bnd in [(0, 2), (2, 7), (7, 12), (12, KC)]:
                nc.sync.dma_start(ht0[:, a:bnd, :], h4[:, 0, a:bnd, :])
                nc.sync.dma_start(wk_sb[:, a:bnd, :], wk4[:, a:bnd, :])
                nc.sync.dma_start(wv_sb[:, a:bnd, :], wv4[:, a:bnd, :])
                nc.sync.dma_start(wq_sb[:, a:bnd, :], wq4[:, a:bnd, :])
            nc.sync.dma_start(cos_sb[:], cosT[:, :])
            nc.sync.dma_start(sin_sb[:], sinT[:, :])
            nc.sync.dma_start(wo_sb[:], wo4[:, :, :])

            # small one-time SBUF setup (gpsimd) + PE warm-up during DMA wait
            make_identity(nc, ident_bf[:])
            nc.gpsimd.memset(wtile[:], 0.0)
            nc.gpsimd.memset(negb[:], -SOFTCAP)
            nc.gpsimd.memset(vE_sb[:, :, D], 1.0)
            # transposed-orientation additive masks (0 keep / -3 mask):
            # value(p=dk, head j, f=dq-in-pair) independent of head (coef 0).
            for m_t, base, csign in (
                (mA, -1, 1),     # oldest block: edge for half A, all-mask B
                (mB, 127, 1),    # second block: keep A, edge for half B
                (mC, 0, -1),     # diag block of half A; keep for half B
                (mD, -128, -1),  # diag block of half B; all-mask for half A
            ):
                nc.gpsimd.memset(m_t[:], 0.0)
                nc.gpsimd.affine_select(
                    out=m_t[:], in_=m_t[:],
                    compare_op=mybir.AluOpType.is_ge, fill=-3.0,
                    base=base, pattern=[[0, 2], [-csign, 256]],
                    channel_multiplier=csign)
            warm_ps = psA.tile([P, 512], f32, tag="bank", name="warm")
            for i in range(20):
                nc.tensor.matmul(warm_ps[:, 0:P], wtile[:], wtile[:],
                                 start=True, stop=True)

            def rope_pair(ps_lo, ps_hi, dst, m_lo, m_hi, ts):
                tsl = slice(ts * TCW, (ts + 1) * TCW)
                cs, sn = cos_sb[:, tsl], sin_sb[:, tsl]
                t1 = work.tile([P, TCW], f16, tag="rope_t1")
                t2 = work.tile([P, TCW], f16, tag="rope_t2")
                nc.vector.tensor_mul(t1[:], ps_hi[:], sn)
                nc.vector.tensor_mul(t2[:], ps_lo[:], sn)
                lo = dst[:, m_lo, tsl]
                hi = dst[:, m_hi, tsl]
                nc.vector.tensor_mul(lo, ps_lo[:], cs)
                nc.vector.tensor_sub(lo, lo, t1[:])
                nc.vector.tensor_mul(hi, ps_hi[:], cs)
                nc.vector.tensor_add(hi, hi, t2[:])

            def proj_chunk0(ht):
                """ts=0 projection, k-outer: all 8 PSUM groups accumulate in
                lockstep with DMA chunk arrival (Q pair 1 in a second pass)."""
                pq0 = [psA.tile([P, 512], f32, tag="bank", name=f"pq0_{i}")
                       for i in range(2)]
                pk = [psA.tile([P, 512], f32, tag="bank", name=f"pk0_{i}")
                      for i in range(2)]
                pv4 = [psA.tile([P, 512], f32, tag="bank", name=f"pv0_{mt}")
                       for mt in range(4)]
                # K and V first within each chunk: they match the DMA issue
                # order (ht, wk, wv, wq), so the first matmuls of a round do
                # not wait for that round's wq descriptors.
                for k in range(KC):
                    st, sp = (k == 0), (k == KC - 1)
                    for i in range(2):
                        nc.tensor.matmul(
                            pk[i][:], wk_sb[:, k, i * P:(i + 1) * P],
                            ht[:, k, :], start=st, stop=sp)
                    for mt in range(4):
                        nc.tensor.matmul(
                            pv4[mt][:, :D], ht[:, k, mt * P:(mt + 1) * P],
                            wv_sb[:, k, :], start=st, stop=sp)
                    for i in range(2):
                        nc.tensor.matmul(
                            pq0[i][:], wq_sb[:, k, i * P:(i + 1) * P],
                            ht[:, k, :], start=st, stop=sp)
                rope_pair(pq0[0], pq0[1], qt_sb, 0, 1, 0)
                rope_pair(pk[0], pk[1], kt_sb, 0, 1, 0)
                for mt in range(4):
                    nc.scalar.copy(vE_sb[:, mt, 0:D], pv4[mt][:, :D])
                pq1 = [psA.tile([P, 512], f32, tag="bank", name=f"pq1_{i}")
                       for i in range(2)]
                for i in range(2):
                    for k in range(KC):
                        nc.tensor.matmul(
                            pq1[i][:], wq_sb[:, k, (2 + i) * P:(3 + i) * P],
                            ht[:, k, :], start=(k == 0), stop=(k == KC - 1))
                rope_pair(pq1[0], pq1[1], qt_sb, 2, 3, 0)

            def proj_chunk(ts, ht):
                for pair in range(2):
                    pq = [psA.tile([P, 512], f32, tag="bank",
                                   name=f"pq{ts}_{pair}_{i}") for i in range(2)]
                    for i in range(2):
                        m = 2 * pair + i
                        for k in range(KC):
                            nc.tensor.matmul(
                                pq[i][:], wq_sb[:, k, m * P:(m + 1) * P],
                                ht[:, k, :], start=(k == 0), stop=(k == KC - 1))
                    rope_pair(pq[0], pq[1], qt_sb, 2 * pair, 2 * pair + 1, ts)
                pk = [psA.tile([P, 512], f32, tag="bank", name=f"pk{ts}_{i}")
                      for i in range(2)]
                for i in range(2):
                    for k in range(KC):
                        nc.tensor.matmul(
                            pk[i][:], wk_sb[:, k, i * P:(i + 1) * P],
                            ht[:, k, :], start=(k == 0), stop=(k == KC - 1))
                rope_pair(pk[0], pk[1], kt_sb, 0, 1, ts)
                for mt in range(4):
                    pv = psA.tile([P, 512], f32, tag="bank")
                    for k in range(KC):
                        nc.tensor.matmul(
                            pv[:, :D], ht[:, k, mt * P:(mt + 1) * P],
                            wv_sb[:, k, :], start=(k == 0), stop=(k == KC - 1))
                    nc.scalar.copy(vE_sb[:, ts * 4 + mt, 0:D], pv[:, :D])

            def emit_wo(prev, last=False):
                """Wo partials for finished query blocks [(atT, q0), ...]."""
                if prev is None:
                    return
                for atT, q0 in prev:
                    osb = work.tile([P, HID], f16, tag="osb", name=f"osb{q0}")
                    hg0 = 0
                    for gi, hgw in enumerate(HG_WIDTHS):
                        po = psA.tile([P, 512], f32, tag="bank",
                                      name=f"po{q0}_{gi}")
                        for m in range(4):
                            nc.tensor.matmul(
                                po[:, :hgw], atT[:, m, :],
                                wo_sb[:, m, hg0:hg0 + hgw],
                                start=(m == 0), stop=(m == 3))
                        if gi % 2 == 0:
                            nc.vector.tensor_copy(osb[:, hg0:hg0 + hgw],
                                                  po[:, :hgw])
                        else:
                            nc.scalar.copy(osb[:, hg0:hg0 + hgw], po[:, :hgw])
                        if last:
                            nc.sync.dma_start(out[q0:q0 + P, hg0:hg0 + hgw],
                                              osb[:, hg0:hg0 + hgw])
                        hg0 += hgw
                    if not last:
                        nc.sync.dma_start(out[q0:q0 + P, :], osb[:])

            def emit_scores_pair(p):
                """Transposed softcapped scores for query blocks 2p, 2p+1."""
                j0 = max(0, 2 * p - 8)
                nkb = 2 * p + 2 - j0
                qsl = slice(2 * p * P, (2 * p + 2) * P)
                tbufT = scpool.tile([P, 10, 2, 256], f32, tag="tbufT")
                ptb = ptpool.tile([P, 10, 2, 256], bf16, tag="ptb",
                                  name=f"ptb{p}")
                for jl in range(nkb):
                    kb = j0 + jl
                    ps = psA.tile([P, 2, 256], f32, tag="bank",
                                  name=f"ps{p}_{jl}")
                    psv = ps
                    for h in range(2):
                        for i in range(2):
                            nc.tensor.matmul(
                                psv[:, h, :],
                                kt_sb[:, i, kb * P:(kb + 1) * P],
                                qt_sb[:, 2 * h + i, qsl],
                                start=(i == 0), stop=(i == 1))
                    nc.scalar.activation(
                        tbufT[:, jl, :, :], psv[:, :, :], AF.Tanh,
                        scale=SCALING / SOFTCAP)
                if j0 == 2 * p - 8:
                    nc.vector.tensor_add(tbufT[:, 0], tbufT[:, 0], mA[:])
                    nc.vector.tensor_add(tbufT[:, 1], tbufT[:, 1], mB[:])
                nc.vector.tensor_add(tbufT[:, nkb - 2], tbufT[:, nkb - 2],
                                     mC[:])
                nc.vector.tensor_add(tbufT[:, nkb - 1], tbufT[:, nkb - 1],
                                     mD[:])
                nc.scalar.activation(
                    ptb[:, 0:nkb], tbufT[:, 0:nkb], AF.Exp,
                    scale=SOFTCAP, bias=negb[:])
                return (p, j0, nkb, ptb)

            def finish_pair(pend):
                """AV + normalization + attnT for both halves of a pair."""
                p, j0, nkb, ptb = pend
                at = work.tile([P, 2, 2 * D], bf16, tag="at", name=f"at{p}")
                res = []
                for s in range(2):
                    qi = 2 * p + s
                    kb_lo = max(0, qi - 8)
                    nb = qi - kb_lo + 1
                    for h in range(2):
                        pav = psA.tile([P, 512], f32, tag="bank",
                                       name=f"pav{p}_{s}_{h}")
                        for n in range(nb):
                            kb = kb_lo + n
                            nc.tensor.matmul(
                                pav[:, 0:D + 1],
                                ptb[:, kb - j0, h, s * P:(s + 1) * P],
                                vE_sb[:, kb, :],
                                start=(n == 0), stop=(n == nb - 1))
                        recip = smpool.tile([P, 1], f32, tag="recip")
                        nc.vector.reciprocal(recip[:], pav[:, D:D + 1])
                        nc.vector.tensor_scalar_mul(
                            at[:, s, h * D:(h + 1) * D], pav[:, 0:D],
                            recip[:])
                # both halves' transposes after all AV matmuls, so half A's
                # eviction (DVE) completes under half B's AV stream.
                for s in range(2):
                    qi = 2 * p + s
                    attps = psA.tile([P, 512], bf16, tag="bank",
                                     name=f"att{p}_{s}")
                    for m in range(4):
                        nc.tensor.transpose(
                            attps[:, m * P:(m + 1) * P],
                            at[:, s, m * P:(m + 1) * P], ident_bf[:])
                    atT = att3.tile([P, 4, P], bf16, tag="atT",
                                    name=f"atT{p}_{s}")
                    nc.scalar.copy(atT[:], attps[:])
                    res.append((atT, qi * P))
                return res

            # ---------------- merged pipeline (2-deep) ----------------
            # proj(1) is emitted right after scores(1) (one pair earlier than
            # needed) to pad the pipeline-fill iterations that have no Wo yet.
            prev = None      # finished pair awaiting Wo
            pend = None      # scored pair awaiting softmax/AV
            hts = {0: ht0}
            for p in range(NPAIR):
                if p % 2 == 0 and p // 2 + 1 < NTC:
                    tsn = p // 2 + 1
                    hts[tsn] = hpool.tile([P, KC, TCW], f16, tag="ht",
                                          name=f"ht{tsn}")
                    nc.sync.dma_start(hts[tsn][:], h4[:, tsn, :, :])
                if p == 0:
                    proj_chunk0(hts[0])
                elif p in (4, 6):
                    proj_chunk(p // 2, hts[p // 2])
                sc = emit_scores_pair(p)
                if p == 1:
                    proj_chunk(1, hts[1])
                emit_wo(prev)
                prev = None
                if pend is not None:
                    prev = finish_pair(pend)
                pend = sc
            emit_wo(prev)
            prev = finish_pair(pend)
            emit_wo(prev, last=True)

    nc.compile()
    return nc


def _get_nc():
    if "nc" not in _CACHED:
        _CACHED["nc"] = _build_nc()
    return _CACHED["nc"]


def kernel(hidden_states, Wq, Wk, Wv, Wo, position_ids):
    import ml_dtypes
    from concourse.bass_utils import run_bass_kernel_spmd

    hidden_states = np.asarray(hidden_states)
    Wq, Wk, Wv, Wo = (np.asarray(a) for a in (Wq, Wk, Wv, Wo))
    position_ids = np.asarray(position_ids)

    inv_freq = 1.0 / (THETA ** (np.arange(0, D, 2, dtype=np.float64) / D))
    freqs = position_ids.astype(np.float64)[None, :] * inv_freq[:, None]
    cos_t = np.cos(freqs).astype(np.float16)
    sin_t = np.sin(freqs).astype(np.float16)

    in_maps = []
    for c in range(8):
        b, r = divmod(c, 4)
        hT = hidden_states[b].T.astype(np.float16)          # [HID, S]
        h4 = np.ascontiguousarray(
            hT.reshape(KC, P, NTC, TCW).transpose(1, 2, 0, 3))
        wq4 = np.ascontiguousarray(
            Wq[512 * r:512 * (r + 1)].T.astype(np.float16)
            .reshape(KC, P, 2 * D).transpose(1, 0, 2))
        wk4 = np.ascontiguousarray(
            Wk[256 * r:256 * (r + 1)].T.astype(np.float16)
            .reshape(KC, P, D).transpose(1, 0, 2))
        wv4 = np.ascontiguousarray(
            Wv[256 * r:256 * (r + 1)].T.astype(np.float16)
            .reshape(KC, P, D).transpose(1, 0, 2))
        wo4 = np.ascontiguousarray(
            Wo[:, 512 * r:512 * (r + 1)].T.astype(ml_dtypes.bfloat16)
            .reshape(4, P, HID).transpose(1, 0, 2))
        in_maps.append({
            "h4": h4, "wq4": wq4, "wk4": wk4, "wv4": wv4, "wo4": wo4,
            "cosT": cos_t, "sinT": sin_t,
        })

    _CACHED["last_in_maps"] = in_maps
    globals()["_last_in_maps"] = in_maps
    res = run_bass_kernel_spmd(_get_nc(), in_maps, core_ids=list(range(8)))
    parts = [r["out"].astype(np.float32) for r in res.results]
    full = np.stack([
        parts[0] + parts[1] + parts[2] + parts[3],
        parts[4] + parts[5] + parts[6] + parts[7],
    ])
    return full
